# revision 27
# baseline (speedup 1.0000x reference)
"""Trainium2 Bass kernel for nn_EquivariantProductBasisBlock.

Math: for each node n (species s) and channel c the MACE symmetric
contraction reduces to

    f[n,c,L] = sum_i x[n,c,i] * H[n,c,(L,i)]
    H[n,c,(L,i)] = sum_K G[s][K, c, (L,i)] * phi[n,c,K]

where phi = the 153 symmetric degree<=2 monomials of x~ = [x, 1] (17 dims)
and G = the U (x) W tables contracted over the CG-path axis p (weight-only,
folded on host).  Output y = concat(f0 @ Wlin0, f1 @ Wlin1) / sqrt(C).

Device mapping (8 cores, channel-sharded: 16 of 128 channels per core):
  - phi[K=153, c, n] built on-chip as A.*B products (A/B = pre-gathered
    monomial factor rows, one paired DMA per multiply so every instruction
    stays within the 1-semaphore-wait ISA limit).
  - nodes host-sorted by species; per species window (<=128 nodes):
    PE matmuls H = phi^T G (K=153 contraction, fp16, FWL-friendly),
    DVE multiply (+x) and grouped reduce over i, PE transpose, PE Wlin
    matmul, DMA partial y out.
  - host sums the 8 channel-partials, un-permutes rows, reorders columns.
"""

import numpy as np

import concourse.bass as bass
import concourse.mybir as mybir
import concourse.tile as tile
from concourse import bacc
from concourse.bass_utils import run_bass_kernel_spmd
from concourse.masks import make_identity

# ---- problem constants (hardcoded per spec) ----
N, C, LM, ELEMS = 1024, 128, 16, 10
NL = 4                      # global L rows: block0 (dim1) + block1 (dim3)
NX = 17                     # x~ = [x_0..x_15, 1]
KTOT = NX * (NX + 1) // 2   # 153 sym pair monomials
K0, K1 = 128, KTOT - 128    # partition chunks (128 + 25)
NCORES = 8
CPC = C // NCORES           # channels per core
NPAD = N + 128              # node axis padded so every window can read 128 cols
LIN = NL * LM               # 64 = (L, i) columns streamed per matmul

PHI_DT = mybir.dt.float16
PHI_NP = np.float16
NQBUILD = 8                 # node-slices for the phi-build pipeline

# pair tables: global pair row r -> (j, m), j <= m
_PAIRS = [(j, m) for j in range(NX) for m in range(j, NX)]


def _build_windows(counts):
    """Species-sorted node windows of <=128 nodes: [(elem, start, len)]."""
    wins = []
    a = 0
    for e in range(ELEMS):
        left = int(counts[e])
        while left > 0:
            w = min(left, 128)
            wins.append((e, a, w))
            a += w
            left -= w
    assert a == N
    return wins


def _build_G(inp):
    """G[K, e, c, l, i] fp32: U (x) W fused tables (weight-only folding)."""
    G = np.zeros((KTOT, ELEMS, C, NL, LM), dtype=np.float32)
    pidx = {p: i for i, p in enumerate(_PAIRS)}
    for b, d in enumerate((1, 3)):
        U1 = np.asarray(inp[f"U1_{b}"], np.float32)
        U2 = np.asarray(inp[f"U2_{b}"], np.float32)
        U3 = np.asarray(inp[f"U3_{b}"], np.float32)
        W1 = np.asarray(inp[f"W1_{b}"], np.float32)
        W2 = np.asarray(inp[f"W2_{b}"], np.float32)
        W3 = np.asarray(inp[f"W3_{b}"], np.float32)
        lb = 0 if b == 0 else 1
        A1 = np.einsum("Lip,epc->ecLi", U1, W1, optimize=True)
        G[pidx[(16, 16)], :, :, lb:lb + d, :] += A1
        A2 = np.einsum("Lijp,epc->ecLij", U2, W2, optimize=True)
        for j in range(LM):
            G[pidx[(j, 16)], :, :, lb:lb + d, :] += A2[:, :, :, :, j]
        A3 = np.einsum("Lijmp,epc->ecLijm", U3, W3, optimize=True)
        for j in range(LM):
            for m in range(j, LM):
                if j == m:
                    coef = A3[:, :, :, :, j, j]
                else:
                    coef = A3[:, :, :, :, j, m] + A3[:, :, :, :, m, j]
                G[pidx[(j, m)], :, :, lb:lb + d, :] += coef
    return G


def build_program(windows):
    # Bacc (not raw Bass): its compile() lowers multi-semaphore waits onto
    # InstEventSemaphore chains (TRN2 allows only 1 wait per instruction).
    nc = bacc.Bacc()
    f32 = mybir.dt.float32
    NW = len(windows)
    qn = NPAD // NQBUILD

    # ab*: per node-slice q, [B_q | A_q] paired so one DMA feeds one multiply
    ab0_d = nc.dram_tensor("ab0", [K0, NQBUILD, 2, CPC, qn], PHI_DT,
                           kind="ExternalInput")
    ab1_d = nc.dram_tensor("ab1", [K1, NQBUILD, 2, CPC, qn], PHI_DT,
                           kind="ExternalInput")
    g0_d = nc.dram_tensor("g0", [K0, ELEMS, CPC, LIN], PHI_DT, kind="ExternalInput")
    g1_d = nc.dram_tensor("g1", [K1, ELEMS, CPC, LIN], PHI_DT, kind="ExternalInput")
    xw_d = nc.dram_tensor("xw", [128, NW, CPC, LM], f32, kind="ExternalInput")
    # block-diagonal Wlin: row (32l + c), col (128l + k) = Wlin_l[c, k]/sqrt(C)
    wl_d = nc.dram_tensor("wl", [128, NL * C], f32, kind="ExternalInput")
    y_d = nc.dram_tensor("y", [N, NL * C], f32, kind="ExternalOutput")

    with tile.TileContext(nc) as tc:
        with (
            tc.tile_pool(name="singles", bufs=1) as singles,
            tc.tile_pool(name="abq", bufs=4) as ab_pool,
            tc.tile_pool(name="tmp", bufs=2) as tmp_pool,
            tc.tile_pool(name="fw", bufs=2) as fw_pool,
            tc.tile_pool(name="fts", bufs=2) as fts_pool,
            tc.tile_pool(name="ysb", bufs=3) as ysb_pool,
            tc.tile_pool(name="ph", bufs=2, space="PSUM") as ph_pool,
            tc.tile_pool(name="pt", bufs=2, space="PSUM") as pt_pool,
            tc.tile_pool(name="py", bufs=2, space="PSUM") as py_pool,
        ):
            # ---- resident loads ----
            g0_sb = singles.tile([K0, ELEMS, CPC, LIN], PHI_DT)
            nc.sync.dma_start(out=g0_sb, in_=g0_d[:])
            g1_sb = singles.tile([K1, ELEMS, CPC, LIN], PHI_DT)
            nc.sync.dma_start(out=g1_sb, in_=g1_d[:])
            wl_sb = singles.tile([128, NL * C], f32)
            nc.sync.dma_start(out=wl_sb, in_=wl_d[:])
            xw_sb = singles.tile([128, NW, CPC, LM], f32)
            nc.sync.dma_start(out=xw_sb, in_=xw_d[:])
            ident = singles.tile([128, 128], f32)
            make_identity(nc, ident)

            # ---- build phi = A .* B ----
            phi = (singles.tile([K0, CPC, NPAD], PHI_DT, name="phi0"),
                   singles.tile([K1, CPC, NPAD], PHI_DT, name="phi1"))
            for ci, ab_d in enumerate((ab0_d, ab1_d)):
                kp = K0 if ci == 0 else K1
                for q in range(NQBUILD):
                    nsl = slice(q * qn, (q + 1) * qn)
                    abq = ab_pool.tile([kp, 2, CPC, qn], PHI_DT,
                                       name=f"abq{ci}_{q}", tag="abq")
                    nc.sync.dma_start(out=abq, in_=ab_d[:, q])
                    dst = phi[ci][:, :, nsl]
                    # split the multiplies across DVE and GPSIMD
                    eng = nc.vector if (ci, q) < (1, NQBUILD // 2) else nc.gpsimd
                    eng.tensor_mul(dst, abq[:, 0], abq[:, 1])

            # ---- per-window pipeline ----
            for w, (e, a, wlen) in enumerate(windows):
                ph = ph_pool.tile([128, CPC, NL, LM], f32)  # 2 PSUM banks
                for c in range(CPC):
                    first = c % 8 == 0  # first matmul touching this bank
                    nc.tensor.matmul(
                        ph[:, c], phi[0][:, c, a:a + 128], g0_sb[:, e, c, :],
                        start=first, stop=False)
                    nc.tensor.matmul(
                        ph[:, c], phi[1][:, c, a:a + 128], g1_sb[:, e, c, :],
                        start=False, stop=c % 8 == 7)

                xwv = xw_sb[:, w]
                xw_b = bass.AP(tensor=xwv.tensor, offset=xwv.offset,
                               ap=[list(xwv.ap[0]), list(xwv.ap[1]),
                                   [0, NL], list(xwv.ap[2])])
                tmp = tmp_pool.tile([128, CPC, NL, LM], f32)
                nc.vector.tensor_mul(tmp, ph, xw_b)

                fw = fw_pool.tile([128, 128], f32)  # col = 32*l + c
                nc.vector.memset(fw, 0.0)
                fw_out = bass.AP(tensor=fw.tensor, offset=fw.offset,
                                 ap=[list(fw.ap[0]), [1, CPC], [32, NL]])
                nc.vector.tensor_reduce(out=fw_out, in_=tmp,
                                        axis=mybir.AxisListType.X,
                                        op=mybir.AluOpType.add)

                ftp = pt_pool.tile([128, 128], f32)
                nc.tensor.transpose(ftp, fw, ident)
                fts = fts_pool.tile([128, 128], f32)
                nc.scalar.copy(fts, ftp)

                py = py_pool.tile([128, NL * C], f32)  # one full bank
                nc.tensor.matmul(py, fts, wl_sb, start=True, stop=True)
                ysb = ysb_pool.tile([128, NL * C], f32)
                nc.scalar.copy(ysb, py)
                nc.sync.dma_start(out=y_d[a:a + wlen], in_=ysb[:wlen])
    nc.compile()
    return nc


def prepare(inputs):
    """Host prep: sort by species, build per-core device inputs."""
    x = np.asarray(inputs["x"], np.float32)
    species = np.asarray(inputs["species"])
    order = np.argsort(species, kind="stable")
    xs = x[order]                           # [N, C, 16]
    sp = np.asarray(species)[order]
    counts = np.bincount(sp, minlength=ELEMS)
    windows = _build_windows(counts)
    NW = len(windows)

    # x~T [17, C, NPAD]
    xt = np.zeros((NX, C, NPAD), np.float32)
    xt[:LM, :, :N] = xs.transpose(2, 1, 0)
    xt[LM, :, :N] = 1.0

    G = _build_G(inputs)                    # [K, E, C, 4, 16] fp32

    # per-window x for the final sum_i contraction: [128, NW, C, LM]
    xw_full = np.zeros((128, NW, C, LM), np.float32)
    for w, (e, a, wlen) in enumerate(windows):
        xw_full[:wlen, w] = xs[a:a + wlen]

    s = 1.0 / np.sqrt(np.float32(C))
    wl_full = np.zeros((NL, C, C), np.float32)
    wl_full[0] = np.asarray(inputs["Wlin_0"], np.float32) * s
    wl_full[1:] = np.asarray(inputs["Wlin_1"], np.float32) * s

    a_src = np.array([p[0] for p in _PAIRS], np.int64)
    b_src = np.array([p[1] for p in _PAIRS], np.int64)
    xt16 = xt.astype(PHI_NP)
    qn = NPAD // NQBUILD
    in_maps = []
    for q in range(NCORES):
        cs, ce = q * CPC, (q + 1) * CPC
        Gq = np.ascontiguousarray(
            G[:, :, cs:ce].reshape(KTOT, ELEMS, CPC, LIN)).astype(PHI_NP)
        wl_q = np.zeros((128, NL * C), np.float32)
        for l in range(NL):
            wl_q[32 * l:32 * l + CPC, 128 * l:128 * (l + 1)] = wl_full[l, cs:ce]
        xtq = np.ascontiguousarray(xt16[:, cs:ce])     # [17, CPC, NPAD]
        af = xtq[a_src].reshape(KTOT, CPC, NQBUILD, qn)
        bf = xtq[b_src].reshape(KTOT, CPC, NQBUILD, qn)
        # [K, NQ, 2(B,A), CPC, qn]
        ab = np.stack([bf, af], axis=0).transpose(1, 3, 0, 2, 4)
        in_maps.append({
            "ab0": np.ascontiguousarray(ab[:K0]),
            "ab1": np.ascontiguousarray(ab[K0:]),
            "g0": np.ascontiguousarray(Gq[:K0]),
            "g1": np.ascontiguousarray(Gq[K0:]),
            "xw": np.ascontiguousarray(xw_full[:, :, cs:ce]),
            "wl": wl_q,
        })
    return in_maps, windows, order


def kernel(**inputs):
    in_maps, windows, order = prepare(inputs)
    nc = build_program(windows)
    res = run_bass_kernel_spmd(nc, in_maps, core_ids=list(range(NCORES)))

    yd = np.zeros((N, NL * C), np.float32)
    for r in res.results:
        yd += np.asarray(r["y"], np.float32)

    # columns: [0:128] = L0 @ k ; block1 interleaved 128 + 3k + i
    y = np.empty((N, 512), np.float32)
    y[:, 0:128] = yd[:, 0:128]
    for i in range(3):
        y[:, 128 + i::3] = yd[:, (1 + i) * 128:(2 + i) * 128]

    inv = np.empty_like(order)
    inv[order] = np.arange(N)
    return y[inv]


# revision 28
# speedup vs baseline: 1.0540x; 1.0540x over previous
"""Trainium2 Bass kernel for nn_EquivariantProductBasisBlock.

Math: for each node n (species s) and channel c the MACE symmetric
contraction reduces to

    f[n,c,L] = sum_i x[n,c,i] * H[n,c,(L,i)]
    H[n,c,(L,i)] = sum_K G[s][K, c, (L,i)] * phi[n,c,K]

where phi = the 153 symmetric degree<=2 monomials of x~ = [x, 1] (17 dims)
and G = the U (x) W tables contracted over the CG-path axis p (weight-only,
folded on host).  Output y = concat(f0 @ Wlin0, f1 @ Wlin1) / sqrt(C).

Device mapping (8 cores, channel-sharded: 16 of 128 channels per core):
  - phi[K=153, c, n] built on-chip as A.*B products (A/B = pre-gathered
    monomial factor rows, one paired DMA per multiply so every instruction
    stays within the 1-semaphore-wait ISA limit).
  - nodes host-sorted by species; per species window (<=128 nodes):
    PE matmuls H = phi^T G (K=153 contraction, fp16, FWL-friendly),
    DVE multiply (+x) and grouped reduce over i, PE transpose, PE Wlin
    matmul, DMA partial y out.
  - host sums the 8 channel-partials, un-permutes rows, reorders columns.
"""

import numpy as np

import concourse.bass as bass
import concourse.mybir as mybir
import concourse.tile as tile
from concourse import bacc
from concourse.bass_utils import run_bass_kernel_spmd
from concourse.masks import make_identity

# ---- problem constants (hardcoded per spec) ----
N, C, LM, ELEMS = 1024, 128, 16, 10
NL = 4                      # global L rows: block0 (dim1) + block1 (dim3)
NX = 17                     # x~ = [x_0..x_15, 1]
KTOT = NX * (NX + 1) // 2   # 153 sym pair monomials
K0, K1 = 128, KTOT - 128    # partition chunks (128 + 25)
NCORES = 8
CPC = C // NCORES           # channels per core
NPAD = N + 128              # node axis padded so every window can read 128 cols
LIN = NL * LM               # 64 = (L, i) columns streamed per matmul

PHI_DT = mybir.dt.float16
PHI_NP = np.float16
NQBUILD = 8                 # node-slices for the phi-build pipeline

# pair tables: global pair row r -> (j, m), j <= m
_PAIRS = [(j, m) for j in range(NX) for m in range(j, NX)]


def _build_windows(counts):
    """Species-sorted node windows of <=128 nodes: [(elem, start, len)]."""
    wins = []
    a = 0
    for e in range(ELEMS):
        left = int(counts[e])
        while left > 0:
            w = min(left, 128)
            wins.append((e, a, w))
            a += w
            left -= w
    assert a == N
    return wins


def _build_G(inp):
    """G[K, e, c, l, i] fp32: U (x) W fused tables (weight-only folding)."""
    G = np.zeros((KTOT, ELEMS, C, NL, LM), dtype=np.float32)
    pidx = {p: i for i, p in enumerate(_PAIRS)}
    for b, d in enumerate((1, 3)):
        U1 = np.asarray(inp[f"U1_{b}"], np.float32)
        U2 = np.asarray(inp[f"U2_{b}"], np.float32)
        U3 = np.asarray(inp[f"U3_{b}"], np.float32)
        W1 = np.asarray(inp[f"W1_{b}"], np.float32)
        W2 = np.asarray(inp[f"W2_{b}"], np.float32)
        W3 = np.asarray(inp[f"W3_{b}"], np.float32)
        lb = 0 if b == 0 else 1
        A1 = np.einsum("Lip,epc->ecLi", U1, W1, optimize=True)
        G[pidx[(16, 16)], :, :, lb:lb + d, :] += A1
        A2 = np.einsum("Lijp,epc->ecLij", U2, W2, optimize=True)
        for j in range(LM):
            G[pidx[(j, 16)], :, :, lb:lb + d, :] += A2[:, :, :, :, j]
        A3 = np.einsum("Lijmp,epc->ecLijm", U3, W3, optimize=True)
        for j in range(LM):
            for m in range(j, LM):
                if j == m:
                    coef = A3[:, :, :, :, j, j]
                else:
                    coef = A3[:, :, :, :, j, m] + A3[:, :, :, :, m, j]
                G[pidx[(j, m)], :, :, lb:lb + d, :] += coef
    return G


def build_program(windows):
    # Bacc (not raw Bass): its compile() lowers multi-semaphore waits onto
    # InstEventSemaphore chains (TRN2 allows only 1 wait per instruction).
    nc = bacc.Bacc()
    f32 = mybir.dt.float32
    NW = len(windows)
    qn = NPAD // NQBUILD

    # ab*: per node-slice q, [B_q | A_q] paired so one DMA feeds one multiply
    ab0_d = nc.dram_tensor("ab0", [K0, NQBUILD, 2, CPC, qn], PHI_DT,
                           kind="ExternalInput")
    ab1_d = nc.dram_tensor("ab1", [K1, NQBUILD, 2, CPC, qn], PHI_DT,
                           kind="ExternalInput")
    g0_d = nc.dram_tensor("g0", [K0, ELEMS, CPC, LIN], PHI_DT, kind="ExternalInput")
    g1_d = nc.dram_tensor("g1", [K1, ELEMS, CPC, LIN], PHI_DT, kind="ExternalInput")
    xw_d = nc.dram_tensor("xw", [128, NW, CPC, LM], f32, kind="ExternalInput")
    # block-diagonal Wlin: row (32l + c), col (128l + k) = Wlin_l[c, k]/sqrt(C)
    wl_d = nc.dram_tensor("wl", [128, NL * C], f32, kind="ExternalInput")
    y_d = nc.dram_tensor("y", [N, NL * C], f32, kind="ExternalOutput")

    with tile.TileContext(nc) as tc:
        with (
            tc.tile_pool(name="singles", bufs=1) as singles,
            tc.tile_pool(name="abq", bufs=4) as ab_pool,
            tc.tile_pool(name="tmp", bufs=2) as tmp_pool,
            tc.tile_pool(name="fw", bufs=2) as fw_pool,
            tc.tile_pool(name="fts", bufs=2) as fts_pool,
            tc.tile_pool(name="ysb", bufs=3) as ysb_pool,
            tc.tile_pool(name="ph", bufs=2, space="PSUM") as ph_pool,
            tc.tile_pool(name="pt", bufs=2, space="PSUM") as pt_pool,
            tc.tile_pool(name="py", bufs=2, space="PSUM") as py_pool,
        ):
            # ---- resident loads ----
            g0_sb = singles.tile([K0, ELEMS, CPC, LIN], PHI_DT)
            nc.sync.dma_start(out=g0_sb, in_=g0_d[:])
            g1_sb = singles.tile([K1, ELEMS, CPC, LIN], PHI_DT)
            nc.sync.dma_start(out=g1_sb, in_=g1_d[:])
            wl_sb = singles.tile([128, NL * C], f32)
            nc.sync.dma_start(out=wl_sb, in_=wl_d[:])
            xw_sb = singles.tile([128, NW, CPC, LM], f32)
            nc.sync.dma_start(out=xw_sb, in_=xw_d[:])
            ident = singles.tile([128, 128], f32)
            make_identity(nc, ident)

            # ---- build phi = A .* B ----
            phi = (singles.tile([K0, CPC, NPAD], PHI_DT, name="phi0"),
                   singles.tile([K1, CPC, NPAD], PHI_DT, name="phi1"))
            # q-major interleave: both chunks of node-slice q land before
            # slice q+1, so early windows' matmuls unblock as soon as
            # possible while later slices still stream in.
            for q in range(NQBUILD):
                for ci, ab_d in enumerate((ab0_d, ab1_d)):
                    kp = K0 if ci == 0 else K1
                    nsl = slice(q * qn, (q + 1) * qn)
                    abq = ab_pool.tile([kp, 2, CPC, qn], PHI_DT,
                                       name=f"abq{ci}_{q}", tag="abq")
                    nc.sync.dma_start(out=abq, in_=ab_d[:, q])
                    dst = phi[ci][:, :, nsl]
                    # chunk-1 multiplies ride on the otherwise-idle GPSIMD
                    # (~3.7x slower per op, but off the DVE critical path)
                    eng = nc.gpsimd if (ci == 1 and q >= 1) else nc.vector
                    eng.tensor_mul(dst, abq[:, 0], abq[:, 1])

            # ---- per-window pipeline ----
            for w, (e, a, wlen) in enumerate(windows):
                ph = ph_pool.tile([128, CPC, NL, LM], f32)  # 2 PSUM banks
                for c in range(CPC):
                    first = c % 8 == 0  # first matmul touching this bank
                    nc.tensor.matmul(
                        ph[:, c], phi[0][:, c, a:a + 128], g0_sb[:, e, c, :],
                        start=first, stop=False)
                    nc.tensor.matmul(
                        ph[:, c], phi[1][:, c, a:a + 128], g1_sb[:, e, c, :],
                        start=False, stop=c % 8 == 7)

                xwv = xw_sb[:, w]
                xw_b = bass.AP(tensor=xwv.tensor, offset=xwv.offset,
                               ap=[list(xwv.ap[0]), list(xwv.ap[1]),
                                   [0, NL], list(xwv.ap[2])])
                tmp = tmp_pool.tile([128, CPC, NL, LM], f32)
                nc.vector.tensor_mul(tmp, ph, xw_b)

                fw = fw_pool.tile([128, 128], f32)  # col = 32*l + c
                nc.vector.memset(fw, 0.0)
                fw_out = bass.AP(tensor=fw.tensor, offset=fw.offset,
                                 ap=[list(fw.ap[0]), [1, CPC], [32, NL]])
                nc.vector.tensor_reduce(out=fw_out, in_=tmp,
                                        axis=mybir.AxisListType.X,
                                        op=mybir.AluOpType.add)

                ftp = pt_pool.tile([128, 128], f32)
                nc.tensor.transpose(ftp, fw, ident)
                fts = fts_pool.tile([128, 128], f32)
                nc.scalar.copy(fts, ftp)

                py = py_pool.tile([128, NL * C], f32)  # one full bank
                nc.tensor.matmul(py, fts, wl_sb, start=True, stop=True)
                ysb = ysb_pool.tile([128, NL * C], f32)
                nc.scalar.copy(ysb, py)
                nc.sync.dma_start(out=y_d[a:a + wlen], in_=ysb[:wlen])
    nc.compile()
    return nc


def prepare(inputs):
    """Host prep: sort by species, build per-core device inputs."""
    x = np.asarray(inputs["x"], np.float32)
    species = np.asarray(inputs["species"])
    order = np.argsort(species, kind="stable")
    xs = x[order]                           # [N, C, 16]
    sp = np.asarray(species)[order]
    counts = np.bincount(sp, minlength=ELEMS)
    windows = _build_windows(counts)
    NW = len(windows)

    # x~T [17, C, NPAD]
    xt = np.zeros((NX, C, NPAD), np.float32)
    xt[:LM, :, :N] = xs.transpose(2, 1, 0)
    xt[LM, :, :N] = 1.0

    G = _build_G(inputs)                    # [K, E, C, 4, 16] fp32

    # per-window x for the final sum_i contraction: [128, NW, C, LM]
    xw_full = np.zeros((128, NW, C, LM), np.float32)
    for w, (e, a, wlen) in enumerate(windows):
        xw_full[:wlen, w] = xs[a:a + wlen]

    s = 1.0 / np.sqrt(np.float32(C))
    wl_full = np.zeros((NL, C, C), np.float32)
    wl_full[0] = np.asarray(inputs["Wlin_0"], np.float32) * s
    wl_full[1:] = np.asarray(inputs["Wlin_1"], np.float32) * s

    a_src = np.array([p[0] for p in _PAIRS], np.int64)
    b_src = np.array([p[1] for p in _PAIRS], np.int64)
    xt16 = xt.astype(PHI_NP)
    qn = NPAD // NQBUILD
    in_maps = []
    for q in range(NCORES):
        cs, ce = q * CPC, (q + 1) * CPC
        Gq = np.ascontiguousarray(
            G[:, :, cs:ce].reshape(KTOT, ELEMS, CPC, LIN)).astype(PHI_NP)
        wl_q = np.zeros((128, NL * C), np.float32)
        for l in range(NL):
            wl_q[32 * l:32 * l + CPC, 128 * l:128 * (l + 1)] = wl_full[l, cs:ce]
        xtq = np.ascontiguousarray(xt16[:, cs:ce])     # [17, CPC, NPAD]
        af = xtq[a_src].reshape(KTOT, CPC, NQBUILD, qn)
        bf = xtq[b_src].reshape(KTOT, CPC, NQBUILD, qn)
        # [K, NQ, 2(B,A), CPC, qn]
        ab = np.stack([bf, af], axis=0).transpose(1, 3, 0, 2, 4)
        in_maps.append({
            "ab0": np.ascontiguousarray(ab[:K0]),
            "ab1": np.ascontiguousarray(ab[K0:]),
            "g0": np.ascontiguousarray(Gq[:K0]),
            "g1": np.ascontiguousarray(Gq[K0:]),
            "xw": np.ascontiguousarray(xw_full[:, :, cs:ce]),
            "wl": wl_q,
        })
    return in_maps, windows, order


def kernel(**inputs):
    in_maps, windows, order = prepare(inputs)
    nc = build_program(windows)
    res = run_bass_kernel_spmd(nc, in_maps, core_ids=list(range(NCORES)))

    yd = np.zeros((N, NL * C), np.float32)
    for r in res.results:
        yd += np.asarray(r["y"], np.float32)

    # columns: [0:128] = L0 @ k ; block1 interleaved 128 + 3k + i
    y = np.empty((N, 512), np.float32)
    y[:, 0:128] = yd[:, 0:128]
    for i in range(3):
        y[:, 128 + i::3] = yd[:, (1 + i) * 128:(2 + i) * 128]

    inv = np.empty_like(order)
    inv[order] = np.arange(N)
    return y[inv]
